# revision 18
# baseline (speedup 1.0000x reference)
"""Trainium2 Bass kernel for the bidirectional-LSTM autoencoder problem.

Self-contained: takes FULL inputs (as produced by the problem's setup), runs
SPMD on 8 NeuronCores (vocab-sharded output GEMM, replicated tiny encoder),
returns the FULL output tuple (preds, embs).

Algorithm notes
---------------
* The output logit GEMM (2048x26 @ 26x50257 -> 412MB f32) dominates memory
  traffic; it is sharded over the vocab dim across the 8 cores (no
  collectives needed).  The GEMM inputs run in bf16 (values are O(1), the
  26-term dot keeps the error ~1e-3, far inside the accuracy gate); the
  accumulation and output stay f32.
* The 2-layer bidirectional encoder LSTM (HID=13) is replicated on every
  core and computed with a Jacobi fixed-point iteration over h: each sweep
  recomputes the gate pre-activations from the previous sweep's h sequence
  (a dense matmul), then solves the cell-state linear recurrence
  c_t = sigma(f_t) * c_{t-1} + b_t EXACTLY with the hardware
  tensor_tensor_scan instruction (fp32 internal state).  The h-coupling
  contracts by ~0.1 per sweep.  Intermediate sweeps use bf16 buffers (the
  final f32 sweep contracts their rounding error away), so SWEEPS=4 reaches
  ~1e-4 relative error on h_n -- and the encoder only affects preds row 0.
* Decoder steps i>0 use zero initial state in the reference, so they are
  closed-form and fully parallel; only preds row 0 depends on the encoder.

Layout: gates live in 32-row partition quadrants (Q0=i, Q1=f, Q2=o, Q3=g),
with the forward direction in rows [0:13) and backward in rows [16:29) of
each quadrant; time runs along the free axis.  The backward direction is
stored time-reversed so both directions scan left-to-right in one
instruction.
"""

import ml_dtypes
import numpy as np

import concourse.bass as bass
import concourse.mybir as mybir
import concourse.tile as tile
from concourse import bacc
from concourse.bass_utils import run_bass_kernel_spmd

F32 = mybir.dt.float32
BF16 = mybir.dt.bfloat16
I32 = mybir.dt.int32
AF = mybir.ActivationFunctionType
MULT = mybir.AluOpType.mult
ADD = mybir.AluOpType.add

HID = 13
L = 2048
VOCAB = 50257
IN_DIM = 256
NCORES = 8
VPAD = 50264          # 8 * 6283
VSH = VPAD // NCORES  # 6283
SWEEPS = 3

# torch gate order in the 4H weight rows is (i, f, g, o); our quadrants are
# (Q0=i, Q1=f, Q2=o, Q3=g)
_TR = {0: 0, 1: 13, 2: 39, 3: 26}  # quadrant -> torch row offset


def _gate_cols():
    ms, ds, trs = [], [], []
    for q in range(4):
        for d in range(2):
            for j in range(HID):
                ms.append(32 * q + 16 * d + j)
                ds.append(d)
                trs.append(_TR[q] + j)
    return np.array(ms), np.array(ds), np.array(trs)


_MS, _DS, _TRS = _gate_cols()


def _wih_T(wih, k):
    out = np.zeros((128, 128), np.float32)
    out[:, _MS] = wih[_DS, _TRS, 128 * k:128 * (k + 1)].T
    return out


def _wih_T32(wih, in_rows):
    out = np.zeros((32, 128), np.float32)
    vals = wih[_DS, _TRS, :]
    for dp in range(2):
        feats = in_rows[dp]
        out[16 * dp:16 * dp + 13, _MS] = vals[:, feats].T
    return out


def _whh_T(whh):
    out = np.zeros((32, 128), np.float32)
    vals = whh[_DS, _TRS, :]
    for i, (m, d) in enumerate(zip(_MS, _DS)):
        out[16 * d:16 * d + 13, m] = vals[i]
    return out


def _bias128(b):
    out = np.zeros((128, 1), np.float32)
    out[_MS, 0] = b[_DS, _TRS]
    return out


def _proj_T(w, in_off):
    out = np.zeros((32, 32), np.float32)
    for d in range(2):
        for ds_ in range(2):
            out[16 * d:16 * d + 13, 16 * ds_:16 * ds_ + 13] = \
                w[13 * ds_:13 * ds_ + 13, in_off + 13 * d:in_off + 13 * d + 13].T
    return out


def _state32(v):
    out = np.zeros((32, 1), np.float32)
    out[0:13, 0] = v[0:13]
    out[16:29, 0] = v[13:26]
    return out


def host_prep(inputs):
    f = lambda k: np.asarray(inputs[k], np.float32)
    x = np.asarray(inputs["x"]).astype(np.int64)

    p = {}
    p["xi"] = x.reshape(16, 128).T.astype(np.int32).copy()
    p["xf"] = x[: L - 1].astype(np.float32).reshape(1, L - 1)

    w0 = f("enc_Wih0")
    p["W0T0"] = _wih_T(w0, 0)
    p["W0T1"] = _wih_T(w0, 1)
    p["Wmm0"] = _whh_T(f("enc_Whh0")).astype(ml_dtypes.bfloat16)
    p["bias0"] = _bias128(f("enc_b0"))
    p["Wih1T"] = _wih_T32(f("enc_Wih1"), [np.arange(13), 13 + np.arange(13)])
    p["Wmm1"] = _whh_T(f("enc_Whh1")).astype(ml_dtypes.bfloat16)
    p["bias1"] = _bias128(f("enc_b1"))

    p["P1aT"] = _proj_T(f("proj1_W"), 0)
    p["P1bT"] = _proj_T(f("proj1_W"), 26)
    p["pb1"] = _state32(f("proj1_b"))
    p["P2aT"] = _proj_T(f("proj2_W"), 0)
    p["P2bT"] = _proj_T(f("proj2_W"), 26)
    p["pb2"] = _state32(f("proj2_b"))

    dwih = f("dec_Wih")
    dwhh = f("dec_Whh")
    db = f("dec_b")
    p["decWhhT"] = _whh_T(dwhh).astype(np.float32)
    rest = np.zeros((1, 128), np.float32)
    rest[0, _MS] = dwih[_DS, _TRS, 0]
    p["decRestT"] = rest
    p["bias_s0"] = _bias128(db) - _bias128(dwih[:, :, 0])
    p["bias_r"] = _bias128(db)

    ifwd = np.zeros((128, 128), np.float32)
    ibwd = np.zeros((128, 128), np.float32)
    for q in range(4):
        for j in range(16):
            ifwd[32 * q + j, 32 * q + j] = 1.0
            ibwd[32 * q + 16 + j, 32 * q + 16 + j] = 1.0
    p["I128f"] = ifwd
    p["I128b"] = ibwd
    scl = np.ones((128, 1), np.float32)
    scl[96:128] = 2.0
    p["scale128"] = scl
    for k in ("bias0", "bias1", "bias_r"):
        p[k] = p[k].copy()
        p[k][96:128] *= 2.0
    p["I128"] = np.eye(128, dtype=np.float32)
    p["ident"] = np.eye(128, dtype=np.float32)

    ow = f("out_W")
    ob = f("out_b")
    owt = np.zeros((33, VPAD), np.float32)
    owt[0:13, :VOCAB] = ow[:, 0:13].T
    owt[16:29, :VOCAB] = ow[:, 13:26].T
    owt[32, :VOCAB] = ob
    p["outWT_full"] = owt.astype(ml_dtypes.bfloat16)
    return p


def build_program():
    nc = bacc.Bacc("TRN2", target_bir_lowering=False)

    dp = lambda name, shape, dt=F32: nc.declare_dram_parameter(name, list(shape), dt, isOutput=False)
    emb_in = dp("emb_W", (VOCAB + 1, IN_DIM))
    xi_in = dp("xi", (128, 16), I32)
    xf_in = dp("xf", (1, L - 1))
    w0t0_in = dp("W0T0", (128, 128))
    w0t1_in = dp("W0T1", (128, 128))
    wmm0_in = dp("Wmm0", (32, 128), BF16)
    bias0_in = dp("bias0", (128, 1))
    wih1t_in = dp("Wih1T", (32, 128))
    wmm1_in = dp("Wmm1", (32, 128), BF16)
    bias1_in = dp("bias1", (128, 1))
    p1a_in = dp("P1aT", (32, 32))
    p1b_in = dp("P1bT", (32, 32))
    pb1_in = dp("pb1", (32, 1))
    p2a_in = dp("P2aT", (32, 32))
    p2b_in = dp("P2bT", (32, 32))
    pb2_in = dp("pb2", (32, 1))
    dwhh_in = dp("decWhhT", (32, 128))
    drest_in = dp("decRestT", (1, 128))
    bs0_in = dp("bias_s0", (128, 1))
    br_in = dp("bias_r", (128, 1))
    i128_in = dp("I128", (128, 128))
    i128f_in = dp("I128f", (128, 128))
    i128b_in = dp("I128b", (128, 128))
    scl_in = dp("scale128", (128, 1))
    ident_in = dp("ident", (128, 128))
    owt_in = dp("outWT", (33, VSH), BF16)

    preds_out = nc.declare_dram_parameter("preds", [L, VSH], BF16, isOutput=True)
    embs_out = nc.declare_dram_parameter("embs", [L, IN_DIM], F32, isOutput=True)

    with tile.TileContext(nc) as tc:
        with (
            tc.tile_pool(name="const", bufs=1) as cpool,
            tc.tile_pool(name="work", bufs=1) as wpool,
            tc.tile_pool(name="stage", bufs=3) as spool,
            tc.tile_pool(name="zbig", bufs=1, space="PSUM") as zpool,
            tc.tile_pool(name="pgemm", bufs=2, space="PSUM") as gpool,
        ):
            def cload(ap_in, shape, dt=F32):
                t = cpool.tile(list(shape), dt, tag=ap_in.name)
                nc.sync.dma_start(t[:], ap_in[:])
                return t

            xi = cload(xi_in, (128, 16), I32)
            xf = cload(xf_in, (1, L - 1))
            W0T0 = cload(w0t0_in, (128, 128))
            W0T1 = cload(w0t1_in, (128, 128))
            Wmm0 = cload(wmm0_in, (32, 128), BF16)
            bias0 = cload(bias0_in, (128, 1))
            Wih1T = cload(wih1t_in, (32, 128))
            Wmm1 = cload(wmm1_in, (32, 128), BF16)
            bias1 = cload(bias1_in, (128, 1))
            P1aT = cload(p1a_in, (32, 32))
            P1bT = cload(p1b_in, (32, 32))
            pb1 = cload(pb1_in, (32, 1))
            P2aT = cload(p2a_in, (32, 32))
            P2bT = cload(p2b_in, (32, 32))
            pb2 = cload(pb2_in, (32, 1))
            decWhhT = cload(dwhh_in, (32, 128))
            decRestT = cload(drest_in, (1, 128))
            bias_s0 = cload(bs0_in, (128, 1))
            bias_r = cload(br_in, (128, 1))
            I128 = cload(i128_in, (128, 128))
            I128f = cload(i128f_in, (128, 128))
            I128b = cload(i128b_in, (128, 128))
            scale128 = cload(scl_in, (128, 1))
            ident = cload(ident_in, (128, 128))
            outWT = cload(owt_in, (33, VSH), BF16)

            embs = wpool.tile([128, 16 * IN_DIM], F32, tag="embs")
            embsT0 = wpool.tile([128, L], F32, tag="embsT0")
            embsT1 = wpool.tile([128, L], F32, tag="embsT1")
            GS = wpool.tile([128, L], F32, tag="GS")
            # f32 buffers (final sweep + decoder-rest)
            S = wpool.tile([128, L], F32, tag="S")
            TG = wpool.tile([32, L], F32, tag="TG")
            W1 = wpool.tile([96, L], F32, tag="W1")
            W2 = wpool.tile([64, L], F32, tag="W2")
            # bf16 buffers (intermediate sweeps)
            Sb = wpool.tile([128, L], BF16, tag="Sb")
            TGb = wpool.tile([32, L], BF16, tag="TGb")
            W1b = wpool.tile([96, L], BF16, tag="W1b")
            Hb = wpool.tile([32, L], BF16, tag="Hb")
            Hfin0 = wpool.tile([32, L], F32, tag="Hfin0")
            Hfin1 = wpool.tile([32, L], F32, tag="Hfin1")
            H0true = wpool.tile([32, L], F32, tag="H0true")
            D = wpool.tile([33, L], BF16, tag="D")
            cn0 = wpool.tile([32, 1], F32, tag="cn0")
            cn1 = wpool.tile([32, 1], F32, tag="cn1")
            sh_sb = wpool.tile([32, 1], F32, tag="sh_sb")
            sc_sb = wpool.tile([32, 1], F32, tag="sc_sb")
            s0 = wpool.tile([96, 1], F32, tag="s0")
            t0 = wpool.tile([32, 1], F32, tag="t0")
            w01 = wpool.tile([96, 1], F32, tag="w01")
            w02 = wpool.tile([64, 1], F32, tag="w02")

            nc.gpsimd.memset(H0true[:], 0.0)
            nc.gpsimd.memset(D[32:33, :], 1.0)
            nc.gpsimd.memset(D[0:32, 0:1], 0.0)

            copy_flip = [0]

            def copy_any(out_ap, in_ap):
                if copy_flip[0] % 2 == 0:
                    nc.vector.tensor_copy(out_ap, in_ap)
                else:
                    nc.scalar.copy(out_ap, in_ap)
                copy_flip[0] += 1

            # =========== decoder rest rows (independent of encoder) ===========
            for (h0, h1) in [(0, 1024), (1024, L - 1)]:
                Zr = zpool.tile([128, 1024], F32, tag="Z", space="PSUM")
                for blk in range((h1 - h0 + 511) // 512):
                    c0 = h0 + 512 * blk
                    c1 = min(c0 + 512, h1)
                    nc.tensor.matmul(Zr[:, c0 - h0:c1 - h0], lhsT=decRestT[:],
                                     rhs=xf[:, c0:c1], start=True, stop=True)
                nc.scalar.activation(S[:, h0:h1], Zr[:, : h1 - h0], AF.Sigmoid,
                                     bias=bias_r[:, :1], scale=scale128[:, :1])
                nc.vector.tensor_scalar(out=TG[:, h0:h1], in0=S[96:128, h0:h1],
                                        scalar1=2.0, scalar2=-1.0,
                                        op0=MULT, op1=ADD)
                nc.vector.tensor_tensor(out=W2[32:64, h0:h1], in0=S[0:32, h0:h1],
                                        in1=TG[:, h0:h1], op=MULT)
                nc.scalar.activation(W1[64:96, h0:h1], W2[32:64, h0:h1], AF.Tanh)
                nc.vector.tensor_tensor(out=D[0:32, h0 + 1:h1 + 1], in0=S[64:96, h0:h1],
                                        in1=W1[64:96, h0:h1], op=MULT)

            # =========== GEMM t-block machinery (bf16 in/out, f32 accum) =====
            CHUNKS = [(0, 4096), (4096, VSH)]

            def gemm_tblock(tb):
                base = 128 * tb
                for (c0, c1) in CHUNKS:
                    st = spool.tile([128, 4096], BF16, tag="stage")
                    n0 = c0
                    while n0 < c1:
                        n1 = min(n0 + 1024, c1)
                        pg = gpool.tile([128, 1024], F32, tag="pg", space="PSUM")
                        m0 = n0
                        while m0 < n1:
                            m1 = min(m0 + 512, n1)
                            nc.tensor.matmul(pg[:, m0 - n0:m1 - n0],
                                             lhsT=D[:, base:base + 128],
                                             rhs=outWT[:, m0:m1],
                                             start=True, stop=True)
                            m0 = m1
                        # evacuation 2:1 toward ACT (DVE carries the scan)
                        if copy_flip[0] % 3 < 2:
                            nc.scalar.copy(st[:, n0 - c0:n1 - c0], pg[:, : n1 - n0])
                        else:
                            nc.vector.tensor_copy(st[:, n0 - c0:n1 - c0], pg[:, : n1 - n0])
                        copy_flip[0] += 1
                        n0 = n1
                    nc.sync.dma_start(out=preds_out[base:base + 128, c0:c1],
                                      in_=st[:, : c1 - c0])

            # head-fill: t-blocks 0..4 while the gather/transpose phase runs
            # (t-block 0's row 0 is garbage here; a row-0-only pass at the end
            # overwrites it once the encoder is done)
            for tb in range(0, 5):
                gemm_tblock(tb)

            # =========== embedding gather + embs output ===========
            for j in range(16):
                nc.gpsimd.indirect_dma_start(
                    out=embs[:, j * IN_DIM:(j + 1) * IN_DIM],
                    out_offset=None,
                    in_=emb_in[:],
                    in_offset=bass.IndirectOffsetOnAxis(ap=xi[:, j:j + 1], axis=0),
                )
            embs_v = embs[:].rearrange("p (j d) -> p j d", d=IN_DIM)
            out_v = embs_out[:].rearrange("(j p) d -> p j d", p=128)
            nc.sync.dma_start(out=out_v, in_=embs_v)

            # =========== transpose embs -> embsT ===========
            for tb in range(16):
                for dh in range(2):
                    pt = zpool.tile([128, 128], F32, tag="ptr", space="PSUM")
                    nc.tensor.transpose(
                        pt[:],
                        embs[:, tb * IN_DIM + dh * 128: tb * IN_DIM + dh * 128 + 128],
                        ident[:],
                    )
                    dst = embsT0 if dh == 0 else embsT1
                    copy_any(dst[:, tb * 128:(tb + 1) * 128], pt[:])

            # =========== G for layer 0 ===========
            def g_half_copies(Gpt, h0, h1):
                """PSUM G half [h0:h1) -> GS stored FULLY time-reversed (both
                dirs); the per-sweep masked-identity matmuls un-reverse the fwd
                rows on the fly."""
                for q in range(4):
                    r0 = 32 * q
                    copy_any(GS[r0:r0 + 32, L - h1:L - h0],
                             Gpt[r0:r0 + 32, h1 - h0 - 1::-1])

            for (h0, h1) in [(0, 1024), (1024, L)]:
                Gp = zpool.tile([128, 1024], F32, tag="Z", space="PSUM")
                for blk in range(2):
                    c0 = h0 + 512 * blk
                    nc.tensor.matmul(Gp[:, 512 * blk:512 * blk + 512], lhsT=W0T0[:],
                                     rhs=embsT0[:, c0:c0 + 512],
                                     start=True, stop=False)
                for blk in range(2):
                    c0 = h0 + 512 * blk
                    nc.tensor.matmul(Gp[:, 512 * blk:512 * blk + 512], lhsT=W0T1[:],
                                     rhs=embsT1[:, c0:c0 + 512],
                                     start=False, stop=True)
                g_half_copies(Gp, h0, h1)

            # =========== encoder sweeps ===========
            def sweeps(Wmm, bias, Hfin):
                """Jacobi sweeps: SWEEPS-1 in bf16, final in f32 -> Hfin.

                Each sweep is split into two column halves so ACT/DVE stages of
                consecutive halves pipeline; the c-scan chains via initial=."""
                HALVES = [(0, 1024), (1024, 2048)]
                for k in range(SWEEPS):
                    fin = k == SWEEPS - 1
                    S_, TG_, W1_ = (S, TG, W1) if fin else (Sb, TGb, W1b)
                    Hout = Hfin if fin else Hb
                    for (h0, h1) in HALVES:
                        Zp = zpool.tile([128, 1024], F32, tag="Z", space="PSUM")
                        for blk in range(2):
                            g0 = h0 + 512 * blk
                            # bwd rows: GS straight (already in scan time)
                            nc.tensor.matmul(Zp[:, 512 * blk:512 * (blk + 1)],
                                             lhsT=I128b[:],
                                             rhs=GS[:, g0:g0 + 512],
                                             start=True, stop=False)
                            # fwd rows: un-reverse via reversed rhs AP
                            lo = L - g0 - 512
                            rev = GS[:, L - 1 - g0:(lo - 1 if lo > 0 else None):-1]
                            nc.tensor.matmul(Zp[:, 512 * blk:512 * (blk + 1)],
                                             lhsT=I128f[:],
                                             rhs=rev,
                                             start=False, stop=(k == 0))
                        if k > 0:
                            for blk in range(2):
                                g0 = max(h0 + 512 * blk, 1)
                                g1 = h0 + 512 * (blk + 1)
                                nc.tensor.matmul(Zp[:, g0 - h0:g1 - h0],
                                                 lhsT=Wmm[:],
                                                 rhs=Hb[:, g0 - 1:g1 - 1],
                                                 start=False, stop=True)
                        nc.scalar.activation(S_[:, h0:h1], Zp[:, : h1 - h0], AF.Sigmoid,
                                             bias=bias[:, :1], scale=scale128[:, :1])
                        nc.vector.tensor_scalar(out=TG_[:, h0:h1], in0=S_[96:128, h0:h1],
                                                scalar1=2.0, scalar2=-1.0,
                                                op0=MULT, op1=ADD)
                        nc.vector.tensor_tensor(out=W1_[32:64, h0:h1], in0=S_[0:32, h0:h1],
                                                in1=TG_[:, h0:h1], op=MULT)
                        init = 0.0 if h0 == 0 else W2[32:64, h0 - 1:h0]
                        nc.vector.tensor_tensor_scan(out=W2[32:64, h0:h1],
                                                     data0=S_[32:64, h0:h1],
                                                     data1=W1_[32:64, h0:h1],
                                                     initial=init,
                                                     op0=MULT, op1=ADD)
                        nc.scalar.activation(W1_[64:96, h0:h1], W2[32:64, h0:h1], AF.Tanh)
                        nc.vector.tensor_tensor(out=Hout[0:32, h0:h1], in0=S_[64:96, h0:h1],
                                                in1=W1_[64:96, h0:h1], op=MULT)
                        yield k

            tb_iter = iter(range(5, 16))

            def emit_tblocks(n):
                for _ in range(n):
                    tb = next(tb_iter, None)
                    if tb is not None:
                        gemm_tblock(tb)

            # ---- layer 0 ----
            for k in sweeps(Wmm0, bias0, Hfin0):
                emit_tblocks(1)
            nc.vector.tensor_copy(cn0[:], W2[32:64, L - 1:L])
            nc.vector.tensor_copy(H0true[0:32, :], Hfin0[0:32, L - 1::-1])
            nc.vector.tensor_copy(H0true[0:13, :], Hfin0[0:13, :])

            # ---- G for layer 1 ----
            for (h0, h1) in [(0, 1024), (1024, L)]:
                Gp1 = zpool.tile([128, 1024], F32, tag="Z", space="PSUM")
                for blk in range(2):
                    c0 = h0 + 512 * blk
                    nc.tensor.matmul(Gp1[:, 512 * blk:512 * blk + 512], lhsT=Wih1T[:],
                                     rhs=H0true[:, c0:c0 + 512],
                                     start=True, stop=True)
                g_half_copies(Gp1, h0, h1)

            # ---- layer 1 ----
            li = [0]
            for k in sweeps(Wmm1, bias1, Hfin1):
                li[0] += 1
                if li[0] <= 4:
                    emit_tblocks(1)
            nc.vector.tensor_copy(cn1[:], W2[32:64, L - 1:L])
            emit_tblocks(16)  # remaining t-blocks overlap proj/step0/row0

            # =========== projections -> dec initial state ===========
            ph = zpool.tile([32, 1], F32, tag="ptr", space="PSUM")
            nc.tensor.matmul(ph[:], lhsT=P1aT[:], rhs=Hfin0[:, L - 1:L],
                             start=True, stop=False)
            nc.tensor.matmul(ph[:], lhsT=P1bT[:], rhs=Hfin1[:, L - 1:L],
                             start=False, stop=True)
            nc.scalar.activation(sh_sb[:], ph[:], AF.Identity, bias=pb1[:, :1])
            pc = zpool.tile([32, 1], F32, tag="ptr", space="PSUM")
            nc.tensor.matmul(pc[:], lhsT=P2aT[:], rhs=cn0[:],
                             start=True, stop=False)
            nc.tensor.matmul(pc[:], lhsT=P2bT[:], rhs=cn1[:],
                             start=False, stop=True)
            nc.scalar.activation(sc_sb[:], pc[:], AF.Identity, bias=pb2[:, :1])

            # =========== decoder step 0 ===========
            z0 = zpool.tile([128, 1], F32, tag="ptr", space="PSUM")
            nc.tensor.matmul(z0[:], lhsT=decWhhT[:], rhs=sh_sb[:],
                             start=True, stop=True)
            nc.scalar.activation(s0[:], z0[0:96, :], AF.Sigmoid,
                                 bias=bias_s0[0:96, :1])
            nc.scalar.activation(t0[:], z0[96:128, :], AF.Tanh,
                                 bias=bias_s0[96:128, :1])
            nc.vector.tensor_tensor(out=w01[32:64, :], in0=s0[0:32, :],
                                    in1=t0[:], op=MULT)
            nc.vector.tensor_copy(w02[32:64, :], sc_sb[:])
            nc.vector.tensor_tensor(out=w02[32:64, :], in0=s0[32:64, :],
                                    in1=w02[32:64, :], op=MULT)
            nc.vector.tensor_tensor(out=w01[32:64, :], in0=w02[32:64, :],
                                    in1=w01[32:64, :], op=ADD)
            nc.scalar.activation(w01[64:96, :], w01[32:64, :], AF.Tanh)
            nc.vector.tensor_tensor(out=D[0:32, 0:1], in0=s0[64:96, :],
                                    in1=w01[64:96, :], op=MULT)

            # =========== final row-0-only GEMM (t = 0) ===========
            str0 = spool.tile([1, VSH], BF16, tag="str0")
            n0 = 0
            while n0 < VSH:
                n1 = min(n0 + 512, VSH)
                pg = gpool.tile([128, 512], F32, tag="pg", space="PSUM")
                nc.tensor.matmul(pg[0:1, : n1 - n0], lhsT=D[:, 0:1],
                                 rhs=outWT[:, n0:n1], start=True, stop=True)
                copy_any(str0[:, n0:n1], pg[0:1, : n1 - n0])
                n0 = n1
            nc.sync.dma_start(out=preds_out[0:1, :], in_=str0[:])

    nc.compile()
    return nc


_NC_CACHE = {}


def _get_program():
    if "nc" not in _NC_CACHE:
        _NC_CACHE["nc"] = build_program()
    return _NC_CACHE["nc"]


def make_in_maps(inputs):
    p = host_prep(inputs)
    emb_W = np.ascontiguousarray(np.asarray(inputs["emb_W"], np.float32))
    common = {
        "emb_W": emb_W, "xi": p["xi"], "xf": p["xf"],
        "W0T0": p["W0T0"], "W0T1": p["W0T1"], "Wmm0": p["Wmm0"], "bias0": p["bias0"],
        "Wih1T": p["Wih1T"], "Wmm1": p["Wmm1"], "bias1": p["bias1"],
        "P1aT": p["P1aT"], "P1bT": p["P1bT"], "pb1": p["pb1"],
        "P2aT": p["P2aT"], "P2bT": p["P2bT"], "pb2": p["pb2"],
        "decWhhT": p["decWhhT"], "decRestT": p["decRestT"],
        "bias_s0": p["bias_s0"], "bias_r": p["bias_r"],
        "I128": p["I128"], "ident": p["ident"], "scale128": p["scale128"],
        "I128f": p["I128f"], "I128b": p["I128b"],
    }
    owt = p["outWT_full"]
    return [
        {**common, "outWT": np.ascontiguousarray(owt[:, c * VSH:(c + 1) * VSH])}
        for c in range(NCORES)
    ]


def kernel(**inputs):
    in_maps = make_in_maps(inputs)
    nc = _get_program()
    res = run_bass_kernel_spmd(nc, in_maps, core_ids=list(range(NCORES)))
    preds = np.concatenate(
        [r["preds"].astype(np.float32) for r in res.results], axis=1)[:, :VOCAB]
    embs = res.results[0]["embs"]
    return preds, embs.astype(np.float32, copy=False)


# revision 19
# speedup vs baseline: 1.0891x; 1.0891x over previous
"""Trainium2 Bass kernel for the bidirectional-LSTM autoencoder problem.

Self-contained: takes FULL inputs (as produced by the problem's setup), runs
SPMD on 8 NeuronCores (vocab-sharded output GEMM, replicated tiny encoder),
returns the FULL output tuple (preds, embs).

Algorithm notes
---------------
* The output logit GEMM (2048x26 @ 26x50257 -> 412MB f32) dominates memory
  traffic; it is sharded over the vocab dim across the 8 cores (no
  collectives needed).  The GEMM inputs run in bf16 (values are O(1), the
  26-term dot keeps the error ~1e-3, far inside the accuracy gate); the
  accumulation and output stay f32.
* The 2-layer bidirectional encoder LSTM (HID=13) is replicated on every
  core and computed with a Jacobi fixed-point iteration over h: each sweep
  recomputes the gate pre-activations from the previous sweep's h sequence
  (a dense matmul), then solves the cell-state linear recurrence
  c_t = sigma(f_t) * c_{t-1} + b_t EXACTLY with the hardware
  tensor_tensor_scan instruction (fp32 internal state).  The h-coupling
  contracts by ~0.1 per sweep.  Intermediate sweeps use bf16 buffers (the
  final f32 sweep contracts their rounding error away), so SWEEPS=4 reaches
  ~1e-4 relative error on h_n -- and the encoder only affects preds row 0.
* Decoder steps i>0 use zero initial state in the reference, so they are
  closed-form and fully parallel; only preds row 0 depends on the encoder.

Layout: gates live in 32-row partition quadrants (Q0=i, Q1=f, Q2=o, Q3=g),
with the forward direction in rows [0:13) and backward in rows [16:29) of
each quadrant; time runs along the free axis.  The backward direction is
stored time-reversed so both directions scan left-to-right in one
instruction.
"""

import ml_dtypes
import numpy as np

import concourse.bass as bass
import concourse.mybir as mybir
import concourse.tile as tile
from concourse import bacc
from concourse.bass_utils import run_bass_kernel_spmd

F32 = mybir.dt.float32
BF16 = mybir.dt.bfloat16
I32 = mybir.dt.int32
AF = mybir.ActivationFunctionType
MULT = mybir.AluOpType.mult
ADD = mybir.AluOpType.add

HID = 13
L = 2048
VOCAB = 50257
IN_DIM = 256
NCORES = 8
VPAD = 50264          # 8 * 6283
VSH = VPAD // NCORES  # 6283
SWEEPS = 3

# torch gate order in the 4H weight rows is (i, f, g, o); our quadrants are
# (Q0=i, Q1=f, Q2=o, Q3=g)
_TR = {0: 0, 1: 13, 2: 39, 3: 26}  # quadrant -> torch row offset


def _gate_cols():
    ms, ds, trs = [], [], []
    for q in range(4):
        for d in range(2):
            for j in range(HID):
                ms.append(32 * q + 16 * d + j)
                ds.append(d)
                trs.append(_TR[q] + j)
    return np.array(ms), np.array(ds), np.array(trs)


_MS, _DS, _TRS = _gate_cols()


def _wih_T(wih, k):
    out = np.zeros((128, 128), np.float32)
    out[:, _MS] = wih[_DS, _TRS, 128 * k:128 * (k + 1)].T
    return out


def _wih_T32(wih, in_rows):
    out = np.zeros((32, 128), np.float32)
    vals = wih[_DS, _TRS, :]
    for dp in range(2):
        feats = in_rows[dp]
        out[16 * dp:16 * dp + 13, _MS] = vals[:, feats].T
    return out


def _whh_T(whh):
    out = np.zeros((32, 128), np.float32)
    vals = whh[_DS, _TRS, :]
    for i, (m, d) in enumerate(zip(_MS, _DS)):
        out[16 * d:16 * d + 13, m] = vals[i]
    return out


def _bias128(b):
    out = np.zeros((128, 1), np.float32)
    out[_MS, 0] = b[_DS, _TRS]
    return out


def _proj_T(w, in_off):
    out = np.zeros((32, 32), np.float32)
    for d in range(2):
        for ds_ in range(2):
            out[16 * d:16 * d + 13, 16 * ds_:16 * ds_ + 13] = \
                w[13 * ds_:13 * ds_ + 13, in_off + 13 * d:in_off + 13 * d + 13].T
    return out


def _state32(v):
    out = np.zeros((32, 1), np.float32)
    out[0:13, 0] = v[0:13]
    out[16:29, 0] = v[13:26]
    return out


def host_prep(inputs):
    f = lambda k: np.asarray(inputs[k], np.float32)
    x = np.asarray(inputs["x"]).astype(np.int64)

    p = {}
    p["xi"] = x.reshape(16, 128).T.astype(np.int32).copy()
    p["xf"] = x[: L - 1].astype(np.float32).reshape(1, L - 1)

    w0 = f("enc_Wih0")
    p["W0T0"] = _wih_T(w0, 0)
    p["W0T1"] = _wih_T(w0, 1)
    p["Wmm0"] = _whh_T(f("enc_Whh0")).astype(ml_dtypes.bfloat16)
    p["bias0"] = _bias128(f("enc_b0"))
    p["Wih1T"] = _wih_T32(f("enc_Wih1"), [np.arange(13), 13 + np.arange(13)])
    p["Wmm1"] = _whh_T(f("enc_Whh1")).astype(ml_dtypes.bfloat16)
    p["bias1"] = _bias128(f("enc_b1"))

    p["P1aT"] = _proj_T(f("proj1_W"), 0)
    p["P1bT"] = _proj_T(f("proj1_W"), 26)
    p["pb1"] = _state32(f("proj1_b"))
    p["P2aT"] = _proj_T(f("proj2_W"), 0)
    p["P2bT"] = _proj_T(f("proj2_W"), 26)
    p["pb2"] = _state32(f("proj2_b"))

    dwih = f("dec_Wih")
    dwhh = f("dec_Whh")
    db = f("dec_b")
    p["decWhhT"] = _whh_T(dwhh).astype(np.float32)
    rest = np.zeros((1, 128), np.float32)
    rest[0, _MS] = dwih[_DS, _TRS, 0]
    p["decRestT"] = rest
    p["bias_s0"] = _bias128(db) - _bias128(dwih[:, :, 0])
    p["bias_r"] = _bias128(db)

    ifwd = np.zeros((128, 128), np.float32)
    ibwd = np.zeros((128, 128), np.float32)
    for q in range(4):
        for j in range(16):
            ifwd[32 * q + j, 32 * q + j] = 1.0
            ibwd[32 * q + 16 + j, 32 * q + 16 + j] = 1.0
    p["I128f"] = ifwd
    p["I128b"] = ibwd
    scl = np.ones((128, 1), np.float32)
    scl[96:128] = 2.0
    p["scale128"] = scl
    for k in ("bias0", "bias1", "bias_r"):
        p[k] = p[k].copy()
        p[k][96:128] *= 2.0
    p["I128"] = np.eye(128, dtype=np.float32)
    p["ident"] = np.eye(128, dtype=np.float32)

    ow = f("out_W")
    ob = f("out_b")
    owt = np.zeros((33, VPAD), np.float32)
    owt[0:13, :VOCAB] = ow[:, 0:13].T
    owt[16:29, :VOCAB] = ow[:, 13:26].T
    owt[32, :VOCAB] = ob
    p["outWT_full"] = owt.astype(ml_dtypes.bfloat16)
    return p


def build_program():
    nc = bacc.Bacc("TRN2", target_bir_lowering=False)

    dp = lambda name, shape, dt=F32: nc.declare_dram_parameter(name, list(shape), dt, isOutput=False)
    emb_in = dp("emb_W", (VOCAB + 1, IN_DIM))
    xi_in = dp("xi", (128, 16), I32)
    xf_in = dp("xf", (1, L - 1))
    w0t0_in = dp("W0T0", (128, 128))
    w0t1_in = dp("W0T1", (128, 128))
    wmm0_in = dp("Wmm0", (32, 128), BF16)
    bias0_in = dp("bias0", (128, 1))
    wih1t_in = dp("Wih1T", (32, 128))
    wmm1_in = dp("Wmm1", (32, 128), BF16)
    bias1_in = dp("bias1", (128, 1))
    p1a_in = dp("P1aT", (32, 32))
    p1b_in = dp("P1bT", (32, 32))
    pb1_in = dp("pb1", (32, 1))
    p2a_in = dp("P2aT", (32, 32))
    p2b_in = dp("P2bT", (32, 32))
    pb2_in = dp("pb2", (32, 1))
    dwhh_in = dp("decWhhT", (32, 128))
    drest_in = dp("decRestT", (1, 128))
    bs0_in = dp("bias_s0", (128, 1))
    br_in = dp("bias_r", (128, 1))
    i128_in = dp("I128", (128, 128))
    i128f_in = dp("I128f", (128, 128))
    i128b_in = dp("I128b", (128, 128))
    scl_in = dp("scale128", (128, 1))
    ident_in = dp("ident", (128, 128))
    owt_in = dp("outWT", (33, VSH), BF16)

    preds_out = nc.declare_dram_parameter("preds", [L, VSH], BF16, isOutput=True)
    embs_out = nc.declare_dram_parameter("embs", [L, IN_DIM], F32, isOutput=True)

    with tile.TileContext(nc) as tc:
        with (
            tc.tile_pool(name="const", bufs=1) as cpool,
            tc.tile_pool(name="work", bufs=1) as wpool,
            tc.tile_pool(name="stage", bufs=3) as spool,
            tc.tile_pool(name="zbig", bufs=2, space="PSUM") as zpool,
            tc.tile_pool(name="pgemm", bufs=2, space="PSUM") as gpool,
        ):
            def cload(ap_in, shape, dt=F32):
                t = cpool.tile(list(shape), dt, tag=ap_in.name)
                nc.sync.dma_start(t[:], ap_in[:])
                return t

            xi = cload(xi_in, (128, 16), I32)
            xf = cload(xf_in, (1, L - 1))
            W0T0 = cload(w0t0_in, (128, 128))
            W0T1 = cload(w0t1_in, (128, 128))
            Wmm0 = cload(wmm0_in, (32, 128), BF16)
            bias0 = cload(bias0_in, (128, 1))
            Wih1T = cload(wih1t_in, (32, 128))
            Wmm1 = cload(wmm1_in, (32, 128), BF16)
            bias1 = cload(bias1_in, (128, 1))
            P1aT = cload(p1a_in, (32, 32))
            P1bT = cload(p1b_in, (32, 32))
            pb1 = cload(pb1_in, (32, 1))
            P2aT = cload(p2a_in, (32, 32))
            P2bT = cload(p2b_in, (32, 32))
            pb2 = cload(pb2_in, (32, 1))
            decWhhT = cload(dwhh_in, (32, 128))
            decRestT = cload(drest_in, (1, 128))
            bias_s0 = cload(bs0_in, (128, 1))
            bias_r = cload(br_in, (128, 1))
            I128 = cload(i128_in, (128, 128))
            I128f = cload(i128f_in, (128, 128))
            I128b = cload(i128b_in, (128, 128))
            scale128 = cload(scl_in, (128, 1))
            ident = cload(ident_in, (128, 128))
            outWT = cload(owt_in, (33, VSH), BF16)

            embs = wpool.tile([128, 16 * IN_DIM], F32, tag="embs")
            embsT0 = wpool.tile([128, L], F32, tag="embsT0")
            embsT1 = wpool.tile([128, L], F32, tag="embsT1")
            GS = wpool.tile([128, L], F32, tag="GS")
            # f32 buffers (final sweep + decoder-rest)
            S = wpool.tile([128, L], F32, tag="S")
            TG = wpool.tile([32, L], F32, tag="TG")
            W1 = wpool.tile([96, L], F32, tag="W1")
            W2 = wpool.tile([64, L], F32, tag="W2")
            # bf16 buffers (intermediate sweeps)
            Sb = wpool.tile([128, L], BF16, tag="Sb")
            TGb = wpool.tile([32, L], BF16, tag="TGb")
            W1b = wpool.tile([96, L], BF16, tag="W1b")
            Hb = wpool.tile([32, L], BF16, tag="Hb")
            Hfin0 = wpool.tile([32, L], F32, tag="Hfin0")
            Hfin1 = wpool.tile([32, L], F32, tag="Hfin1")
            H0true = wpool.tile([32, L], F32, tag="H0true")
            D = wpool.tile([33, L], BF16, tag="D")
            cn0 = wpool.tile([32, 1], F32, tag="cn0")
            cn1 = wpool.tile([32, 1], F32, tag="cn1")
            sh_sb = wpool.tile([32, 1], F32, tag="sh_sb")
            sc_sb = wpool.tile([32, 1], F32, tag="sc_sb")
            s0 = wpool.tile([96, 1], F32, tag="s0")
            t0 = wpool.tile([32, 1], F32, tag="t0")
            w01 = wpool.tile([96, 1], F32, tag="w01")
            w02 = wpool.tile([64, 1], F32, tag="w02")

            nc.vector.memset(H0true[:], 0.0)
            nc.vector.memset(D[32:33, :], 1.0)
            nc.vector.memset(D[0:32, 0:1], 0.0)

            copy_flip = [0]

            def copy_any(out_ap, in_ap):
                if copy_flip[0] % 2 == 0:
                    nc.vector.tensor_copy(out_ap, in_ap)
                else:
                    nc.scalar.copy(out_ap, in_ap)
                copy_flip[0] += 1

            # =========== decoder rest rows (independent of encoder) ===========
            for (h0, h1) in [(0, 1024), (1024, L - 1)]:
                Zr = zpool.tile([128, 1024], F32, tag="Z", space="PSUM")
                for blk in range((h1 - h0 + 511) // 512):
                    c0 = h0 + 512 * blk
                    c1 = min(c0 + 512, h1)
                    nc.tensor.matmul(Zr[:, c0 - h0:c1 - h0], lhsT=decRestT[:],
                                     rhs=xf[:, c0:c1], start=True, stop=True)
                nc.scalar.activation(S[:, h0:h1], Zr[:, : h1 - h0], AF.Sigmoid,
                                     bias=bias_r[:, :1], scale=scale128[:, :1])
                nc.vector.tensor_scalar(out=TG[:, h0:h1], in0=S[96:128, h0:h1],
                                        scalar1=2.0, scalar2=-1.0,
                                        op0=MULT, op1=ADD)
                nc.vector.tensor_tensor(out=W2[32:64, h0:h1], in0=S[0:32, h0:h1],
                                        in1=TG[:, h0:h1], op=MULT)
                nc.scalar.activation(W1[64:96, h0:h1], W2[32:64, h0:h1], AF.Tanh)
                nc.vector.tensor_tensor(out=D[0:32, h0 + 1:h1 + 1], in0=S[64:96, h0:h1],
                                        in1=W1[64:96, h0:h1], op=MULT)

            # =========== GEMM t-block machinery (bf16 in/out, f32 accum) =====
            CHUNKS = [(0, 4096), (4096, VSH)]

            def gemm_tblock(tb):
                base = 128 * tb
                for (c0, c1) in CHUNKS:
                    st = spool.tile([128, 4096], BF16, tag="stage")
                    n0 = c0
                    while n0 < c1:
                        n1 = min(n0 + 1024, c1)
                        pg = gpool.tile([128, 1024], F32, tag="pg", space="PSUM")
                        m0 = n0
                        while m0 < n1:
                            m1 = min(m0 + 512, n1)
                            nc.tensor.matmul(pg[:, m0 - n0:m1 - n0],
                                             lhsT=D[:, base:base + 128],
                                             rhs=outWT[:, m0:m1],
                                             start=True, stop=True)
                            m0 = m1
                        # evacuation 2:1 toward ACT (DVE carries the scan)
                        if copy_flip[0] % 3 < 2:
                            nc.scalar.copy(st[:, n0 - c0:n1 - c0], pg[:, : n1 - n0])
                        else:
                            nc.vector.tensor_copy(st[:, n0 - c0:n1 - c0], pg[:, : n1 - n0])
                        copy_flip[0] += 1
                        n0 = n1
                    nc.sync.dma_start(out=preds_out[base:base + 128, c0:c1],
                                      in_=st[:, : c1 - c0])

            # head-fill: t-blocks 0..4 while the gather/transpose phase runs
            # (t-block 0's row 0 is garbage here; a row-0-only pass at the end
            # overwrites it once the encoder is done)
            for tb in range(0, 5):
                gemm_tblock(tb)

            # =========== embedding gather + embs output ===========
            for j in range(16):
                nc.gpsimd.indirect_dma_start(
                    out=embs[:, j * IN_DIM:(j + 1) * IN_DIM],
                    out_offset=None,
                    in_=emb_in[:],
                    in_offset=bass.IndirectOffsetOnAxis(ap=xi[:, j:j + 1], axis=0),
                )
            embs_v = embs[:].rearrange("p (j d) -> p j d", d=IN_DIM)
            out_v = embs_out[:].rearrange("(j p) d -> p j d", p=128)
            nc.sync.dma_start(out=out_v, in_=embs_v)

            # =========== transpose embs -> embsT ===========
            for tb in range(16):
                for dh in range(2):
                    pt = zpool.tile([128, 128], F32, tag="Z", space="PSUM")
                    nc.tensor.transpose(
                        pt[:],
                        embs[:, tb * IN_DIM + dh * 128: tb * IN_DIM + dh * 128 + 128],
                        ident[:],
                    )
                    dst = embsT0 if dh == 0 else embsT1
                    copy_any(dst[:, tb * 128:(tb + 1) * 128], pt[:])

            # =========== G for layer 0 ===========
            def g_half_copies(Gpt, h0, h1):
                """PSUM G half [h0:h1) -> GS stored FULLY time-reversed (both
                dirs); the per-sweep masked-identity matmuls un-reverse the fwd
                rows on the fly."""
                for q in range(4):
                    r0 = 32 * q
                    copy_any(GS[r0:r0 + 32, L - h1:L - h0],
                             Gpt[r0:r0 + 32, h1 - h0 - 1::-1])

            for (h0, h1) in [(0, 1024), (1024, L)]:
                Gp = zpool.tile([128, 1024], F32, tag="Z", space="PSUM")
                for blk in range(2):
                    c0 = h0 + 512 * blk
                    nc.tensor.matmul(Gp[:, 512 * blk:512 * blk + 512], lhsT=W0T0[:],
                                     rhs=embsT0[:, c0:c0 + 512],
                                     start=True, stop=False)
                for blk in range(2):
                    c0 = h0 + 512 * blk
                    nc.tensor.matmul(Gp[:, 512 * blk:512 * blk + 512], lhsT=W0T1[:],
                                     rhs=embsT1[:, c0:c0 + 512],
                                     start=False, stop=True)
                g_half_copies(Gp, h0, h1)

            # =========== encoder sweeps ===========
            def sweeps(Wmm, bias, Hfin, tag):
                """Jacobi sweeps: SWEEPS-1 in bf16, final in f32 -> Hfin.

                The two column halves run as DECOUPLED streams: half1 takes its
                c-scan initial and its h_{t-1} boundary column from the
                PREVIOUS sweep (snapshots cb/hb) instead of chaining on this
                sweep's half0 -- boundary errors decay by ~0.55/step inside the
                half and by the Jacobi rate across sweeps."""
                HALVES = [(0, 1024), (1024, 2048)]
                cb_prev, hb_prev = None, None
                for k in range(SWEEPS):
                    fin = k == SWEEPS - 1
                    S_, TG_, W1_ = (S, TG, W1) if fin else (Sb, TGb, W1b)
                    Hout = Hfin if fin else Hb
                    cb = wpool.tile([64, 1], F32, tag=f"cb{tag}{k % 2}")
                    hb = wpool.tile([32, 1], BF16, tag=f"hb{tag}{k % 2}")
                    for (h0, h1) in HALVES:
                        Zp = zpool.tile([128, 1024], F32, tag="Z", space="PSUM")
                        for blk in range(2):
                            g0 = h0 + 512 * blk
                            # bwd rows: GS straight (already in scan time)
                            nc.tensor.matmul(Zp[:, 512 * blk:512 * (blk + 1)],
                                             lhsT=I128b[:],
                                             rhs=GS[:, g0:g0 + 512],
                                             start=True, stop=False)
                            # fwd rows: un-reverse via reversed rhs AP
                            lo = L - g0 - 512
                            rev = GS[:, L - 1 - g0:(lo - 1 if lo > 0 else None):-1]
                            nc.tensor.matmul(Zp[:, 512 * blk:512 * (blk + 1)],
                                             lhsT=I128f[:],
                                             rhs=rev,
                                             start=False, stop=(k == 0))
                        if k > 0:
                            if h0 == 0:
                                nc.tensor.matmul(Zp[:, 1:512], lhsT=Wmm[:],
                                                 rhs=Hb[:, 0:511],
                                                 start=False, stop=True)
                                nc.tensor.matmul(Zp[:, 512:1024], lhsT=Wmm[:],
                                                 rhs=Hb[:, 511:1023],
                                                 start=False, stop=True)
                            else:
                                # boundary column via snapshot from sweep k-1
                                nc.tensor.matmul(Zp[:, 0:1], lhsT=Wmm[:],
                                                 rhs=hb_prev[:],
                                                 start=False, stop=True)
                                nc.tensor.matmul(Zp[:, 1:512], lhsT=Wmm[:],
                                                 rhs=Hb[:, 1024:1535],
                                                 start=False, stop=True)
                                nc.tensor.matmul(Zp[:, 512:1024], lhsT=Wmm[:],
                                                 rhs=Hb[:, 1535:2047],
                                                 start=False, stop=True)
                        nc.scalar.activation(S_[:, h0:h1], Zp[:, : h1 - h0], AF.Sigmoid,
                                             bias=bias[:, :1], scale=scale128[:, :1])
                        nc.vector.tensor_scalar(out=TG_[:, h0:h1], in0=S_[96:128, h0:h1],
                                                scalar1=2.0, scalar2=-1.0,
                                                op0=MULT, op1=ADD)
                        nc.vector.tensor_tensor(out=W1_[32:64, h0:h1], in0=S_[0:32, h0:h1],
                                                in1=TG_[:, h0:h1], op=MULT)
                        if h0 == 0:
                            init = 0.0
                        else:
                            init = 0.0 if cb_prev is None else cb_prev[32:64, 0:1]
                        nc.vector.tensor_tensor_scan(out=W2[32:64, h0:h1],
                                                     data0=S_[32:64, h0:h1],
                                                     data1=W1_[32:64, h0:h1],
                                                     initial=init,
                                                     op0=MULT, op1=ADD)
                        nc.scalar.activation(W1_[64:96, h0:h1], W2[32:64, h0:h1], AF.Tanh)
                        nc.vector.tensor_tensor(out=Hout[0:32, h0:h1], in0=S_[64:96, h0:h1],
                                                in1=W1_[64:96, h0:h1], op=MULT)
                        if h0 == 0 and not fin:
                            # snapshots of the boundary column for sweep k+1
                            nc.vector.tensor_copy(cb[32:64, :], W2[32:64, 1023:1024])
                            nc.vector.tensor_copy(hb[:], Hout[0:32, 1023:1024])
                        yield k
                    cb_prev, hb_prev = cb, hb

            tb_iter = iter(range(5, 16))

            def emit_tblocks(n):
                for _ in range(n):
                    tb = next(tb_iter, None)
                    if tb is not None:
                        gemm_tblock(tb)

            # ---- layer 0 ----
            for k in sweeps(Wmm0, bias0, Hfin0, 0):
                emit_tblocks(1)
            nc.vector.tensor_copy(cn0[:], W2[32:64, L - 1:L])
            nc.vector.tensor_copy(H0true[0:32, :], Hfin0[0:32, L - 1::-1])
            nc.vector.tensor_copy(H0true[0:13, :], Hfin0[0:13, :])

            # ---- G for layer 1 ----
            for (h0, h1) in [(0, 1024), (1024, L)]:
                Gp1 = zpool.tile([128, 1024], F32, tag="Z", space="PSUM")
                for blk in range(2):
                    c0 = h0 + 512 * blk
                    nc.tensor.matmul(Gp1[:, 512 * blk:512 * blk + 512], lhsT=Wih1T[:],
                                     rhs=H0true[:, c0:c0 + 512],
                                     start=True, stop=True)
                g_half_copies(Gp1, h0, h1)

            # ---- layer 1 ----
            li = [0]
            for k in sweeps(Wmm1, bias1, Hfin1, 1):
                li[0] += 1
                if li[0] <= 4:
                    emit_tblocks(1)
            nc.vector.tensor_copy(cn1[:], W2[32:64, L - 1:L])
            emit_tblocks(16)  # remaining t-blocks overlap proj/step0/row0

            # =========== projections -> dec initial state ===========
            ph = zpool.tile([32, 1], F32, tag="Z", space="PSUM")
            nc.tensor.matmul(ph[:], lhsT=P1aT[:], rhs=Hfin0[:, L - 1:L],
                             start=True, stop=False)
            nc.tensor.matmul(ph[:], lhsT=P1bT[:], rhs=Hfin1[:, L - 1:L],
                             start=False, stop=True)
            nc.scalar.activation(sh_sb[:], ph[:], AF.Identity, bias=pb1[:, :1])
            pc = zpool.tile([32, 1], F32, tag="Z", space="PSUM")
            nc.tensor.matmul(pc[:], lhsT=P2aT[:], rhs=cn0[:],
                             start=True, stop=False)
            nc.tensor.matmul(pc[:], lhsT=P2bT[:], rhs=cn1[:],
                             start=False, stop=True)
            nc.scalar.activation(sc_sb[:], pc[:], AF.Identity, bias=pb2[:, :1])

            # =========== decoder step 0 ===========
            z0 = zpool.tile([128, 1], F32, tag="Z", space="PSUM")
            nc.tensor.matmul(z0[:], lhsT=decWhhT[:], rhs=sh_sb[:],
                             start=True, stop=True)
            nc.scalar.activation(s0[:], z0[0:96, :], AF.Sigmoid,
                                 bias=bias_s0[0:96, :1])
            nc.scalar.activation(t0[:], z0[96:128, :], AF.Tanh,
                                 bias=bias_s0[96:128, :1])
            nc.vector.tensor_tensor(out=w01[32:64, :], in0=s0[0:32, :],
                                    in1=t0[:], op=MULT)
            nc.vector.tensor_copy(w02[32:64, :], sc_sb[:])
            nc.vector.tensor_tensor(out=w02[32:64, :], in0=s0[32:64, :],
                                    in1=w02[32:64, :], op=MULT)
            nc.vector.tensor_tensor(out=w01[32:64, :], in0=w02[32:64, :],
                                    in1=w01[32:64, :], op=ADD)
            nc.scalar.activation(w01[64:96, :], w01[32:64, :], AF.Tanh)
            nc.vector.tensor_tensor(out=D[0:32, 0:1], in0=s0[64:96, :],
                                    in1=w01[64:96, :], op=MULT)

            # =========== final row-0-only GEMM (t = 0) ===========
            str0 = spool.tile([1, VSH], BF16, tag="str0")
            n0 = 0
            while n0 < VSH:
                n1 = min(n0 + 512, VSH)
                pg = gpool.tile([128, 512], F32, tag="pg", space="PSUM")
                nc.tensor.matmul(pg[0:1, : n1 - n0], lhsT=D[:, 0:1],
                                 rhs=outWT[:, n0:n1], start=True, stop=True)
                copy_any(str0[:, n0:n1], pg[0:1, : n1 - n0])
                n0 = n1
            nc.sync.dma_start(out=preds_out[0:1, :], in_=str0[:])

    nc.compile()
    return nc


_NC_CACHE = {}


def _get_program():
    if "nc" not in _NC_CACHE:
        _NC_CACHE["nc"] = build_program()
    return _NC_CACHE["nc"]


def make_in_maps(inputs):
    p = host_prep(inputs)
    emb_W = np.ascontiguousarray(np.asarray(inputs["emb_W"], np.float32))
    common = {
        "emb_W": emb_W, "xi": p["xi"], "xf": p["xf"],
        "W0T0": p["W0T0"], "W0T1": p["W0T1"], "Wmm0": p["Wmm0"], "bias0": p["bias0"],
        "Wih1T": p["Wih1T"], "Wmm1": p["Wmm1"], "bias1": p["bias1"],
        "P1aT": p["P1aT"], "P1bT": p["P1bT"], "pb1": p["pb1"],
        "P2aT": p["P2aT"], "P2bT": p["P2bT"], "pb2": p["pb2"],
        "decWhhT": p["decWhhT"], "decRestT": p["decRestT"],
        "bias_s0": p["bias_s0"], "bias_r": p["bias_r"],
        "I128": p["I128"], "ident": p["ident"], "scale128": p["scale128"],
        "I128f": p["I128f"], "I128b": p["I128b"],
    }
    owt = p["outWT_full"]
    return [
        {**common, "outWT": np.ascontiguousarray(owt[:, c * VSH:(c + 1) * VSH])}
        for c in range(NCORES)
    ]


def kernel(**inputs):
    in_maps = make_in_maps(inputs)
    nc = _get_program()
    res = run_bass_kernel_spmd(nc, in_maps, core_ids=list(range(NCORES)))
    preds = np.concatenate(
        [r["preds"].astype(np.float32) for r in res.results], axis=1)[:, :VOCAB]
    embs = res.results[0]["embs"]
    return preds, embs.astype(np.float32, copy=False)
